# revision 26
# baseline (speedup 1.0000x reference)
"""Trainium2 Bass kernel for AttentionFusionModel (B=4, S=4096, D=200).

out = (attn(x1) + attn(x2)) @ Wo.T + bo, with attn sharing Wq/Wk/Wv.

Sharding: (batch, q-half) -> 8 NeuronCores, no collectives. Core 2b+h
computes BOTH modalities of batch b for query rows [h*S/2, (h+1)*S/2),
with full-length K/V for each modality, sums the two normalized attention
outputs locally, and writes its half directly. The host feeds each core
X^T ROTATED by h*S/2 along the sequence axis so every core runs the same
SPMD program (Q always reads columns [0, S/2); softmax over k is
permutation-invariant, so rotated key order changes nothing).

Wo is folded into Wv on the host ((A@V)@Wo == A@(V@Wo); the softmax
row-normalization commutes with the projection), so there is no separate
output-projection stage on device; bo/2 rides each modality's bias row.

Per-core layout strategy (all big matmuls in bf16, fp32 PSUM accumulate):
  X^T [201, S]   fed pre-transposed+bf16+rotated from host (+ones row)
  Q^T [200, S/2], K^T [200, S] = W-stationary matmuls (bias via ones row)
  V2 [S, 201]    = X @ (Wv.T@Wo.T) + (bv@Wo.T + bo/2), natural layout,
                 col 200 = ones (sumexp L)
  scores^T[k,q] tiles = K^T-slice.T @ Q^T   (contract d: 128+72 blocks)
  expT = Exp(scores^T)  on ScalarE (no max subtraction; |scores| ~< 7)
  out_m[q, 201] += expT-slice.T @ V2[k,:]   (et stationary 128x128, V2
                                             streams: full PE utilization)
  out rows = out_0[:, :200]/L_0 + out_1[:, :200]/L_1  (bf16)

A burst of dummy matmuls at kernel start keeps the PE HAM clock-gate warm
(2.4 GHz) through the QKV phase instead of ramping ~50us into the kernel.
All input DMAs go through the Sync-engine dynamic queue: it fans out
across all 16 SDMA engines (~220 GB/s); the GpSimd/Scalar queues do not.
"""

import sys

sys.path.insert(0, "/opt/trn_rl_repo")

import numpy as np
from contextlib import ExitStack

import ml_dtypes

from concourse import bacc, mybir, tile
from concourse.bass_utils import run_bass_kernel_spmd

F32 = mybir.dt.float32
BF16 = mybir.dt.bfloat16
AF = mybir.ActivationFunctionType
ALU = mybir.AluOpType
NP_BF16 = np.dtype(ml_dtypes.bfloat16)

B = 4
S = 4096
D = 200
P = 128
D2 = D - P            # 72
DA = D + 1            # 201 (augmented with ones row / sumexp col)
WCOL = 2 * D + DA     # packed weight columns: wq|wk|wv2
NCORES = 8

NWARM = 70


def _emit(ctx, tc, nc, exts, s_len):
    xt_exts, w_ext, out_ext = exts
    half = s_len // 2
    QG = min(512, half)
    nkb = s_len // P
    nqg = half // QG
    nqb = QG // P

    pers = ctx.enter_context(tc.tile_pool(name="pers", bufs=1))
    qt0 = [pers.tile([P, half], BF16, name=f"qt0_{m}") for m in range(2)]
    qt1 = [pers.tile([D2, half], BF16, name=f"qt1_{m}") for m in range(2)]
    kt0 = [pers.tile([P, s_len], BF16, name=f"kt0_{m}") for m in range(2)]
    kt1 = [pers.tile([D2, s_len], BF16, name=f"kt1_{m}") for m in range(2)]
    v_sb = [pers.tile([P, nkb * DA], BF16, name=f"v_{m}") for m in range(2)]
    w_sb = pers.tile([P, 2 * WCOL], BF16)
    wu = pers.tile([P, 64], BF16)

    # whi = wpack rows 0:128, wlo = wpack rows 128:201 (padded to 128 for the
    # DMA: only 128-partition transfers fan out across all 16 SDMA engines)
    whi = w_sb[:, 0:WCOL]
    wlo = w_sb[0:D2 + 1, WCOL:2 * WCOL]
    wq0, wq1 = whi[:, 0:D], wlo[:, 0:D]
    wk0, wk1 = whi[:, D:2 * D], wlo[:, D:2 * D]
    wv0, wv1 = whi[:, 2 * D:WCOL], wlo[:, 2 * D:WCOL]

    # ---- phase 1: warmup + load + QKV projections (both modalities) ----
    with ExitStack() as ph1:
        wups = ph1.enter_context(tc.tile_pool(name="wups", bufs=1, space="PSUM"))
        nc.vector.memset(wu[:], 0.0)
        trash = wups.tile([P, 64], F32)
        for _ in range(NWARM):
            nc.tensor.matmul(trash[0:64, :], wu[:, 0:64], wu[:],
                             start=True, stop=True)

        xp = ph1.enter_context(tc.tile_pool(name="xp", bufs=1))
        xt0f = [xp.tile([P, s_len], BF16, name=f"xt0_{m}") for m in range(2)]
        xt1f = [xp.tile([P, s_len], BF16, name=f"xt1_{m}") for m in range(2)]
        # matmul views: xt1 rows 0:72 = d 128:200, row 72 = ones
        xt0 = xt0f
        xt1 = [t[0:D2 + 1, :] for t in xt1f]

        nc.sync.dma_start(out=w_sb[:], in_=w_ext[:, :])
        DCH = min(512, s_len)
        for m in range(2):
            for ch in range(s_len // DCH):
                c0, c1 = ch * DCH, (ch + 1) * DCH
                nc.sync.dma_start(out=xt0f[m][:, c0:c1],
                                  in_=xt_exts[m][0:P, c0:c1])
                nc.sync.dma_start(out=xt1f[m][:, c0:c1],
                                  in_=xt_exts[m][P:2 * P, c0:c1])

        qkps = ph1.enter_context(tc.tile_pool(name="qkps", bufs=3, space="PSUM"))
        vps = ph1.enter_context(tc.tile_pool(name="vps", bufs=2, space="PSUM"))
        CH = min(512, s_len)
        qw = min(CH, half)

        def emit_q(m, qch):
            c0, c1 = qch * qw, (qch + 1) * qw
            for ob, obw in ((0, P), (1, D2)):
                tdst = qt0[m] if ob == 0 else qt1[m]
                ps = qkps.tile([P, CH], F32, tag="qk")
                nc.tensor.matmul(ps[0:obw, 0:qw],
                                 wq0[:, ob * P:ob * P + obw],
                                 xt0[m][:, c0:c1], start=True, stop=False)
                nc.tensor.matmul(ps[0:obw, 0:qw],
                                 wq1[:, ob * P:ob * P + obw],
                                 xt1[m][:, c0:c1], start=False, stop=True)
                # big block on DVE, small on ScalarE: the Q tail has no
                # V/K work to hide a 900ns ACT copy behind
                if ob == 0:
                    nc.vector.tensor_copy(tdst[:, c0:c1], ps[0:obw, 0:qw])
                else:
                    nc.scalar.activation(tdst[:, c0:c1], ps[0:obw, 0:qw],
                                         AF.Copy)

        nqch = half // qw
        for m in range(2):
            qdone = 0
            for ch in range(s_len // CH):
                c0, c1 = ch * CH, (ch + 1) * CH
                for ob, obw in ((0, P), (1, D2)):
                    tdst = kt0[m] if ob == 0 else kt1[m]
                    ps = qkps.tile([P, CH], F32, tag="qk")
                    nc.tensor.matmul(ps[0:obw, :], wk0[:, ob * P:ob * P + obw],
                                     xt0[m][:, c0:c1], start=True, stop=False)
                    nc.tensor.matmul(ps[0:obw, :], wk1[:, ob * P:ob * P + obw],
                                     xt1[m][:, c0:c1], start=False, stop=True)
                    if ob == 0:
                        nc.scalar.activation(tdst[:, c0:c1], ps[0:obw, :],
                                             AF.Copy)
                    else:
                        nc.vector.tensor_copy(tdst[:, c0:c1], ps[0:obw, :])
                for n in range(ch * (CH // P), (ch + 1) * (CH // P)):
                    pv = vps.tile([P, DA], F32, tag="pv")
                    nc.tensor.matmul(pv[:], xt0[m][:, n * P:(n + 1) * P], wv0,
                                     start=True, stop=False)
                    nc.tensor.matmul(pv[:], xt1[m][:, n * P:(n + 1) * P], wv1,
                                     start=False, stop=True)
                    nc.vector.tensor_copy(v_sb[m][:, n * DA:(n + 1) * DA],
                                          pv[:])
                # interleave Q (no new data needed) to fill DMA-pacing gaps
                if ch >= 1 and qdone < nqch:
                    emit_q(m, qdone)
                    qdone += 1
            while qdone < nqch:
                emit_q(m, qdone)
                qdone += 1

    # ---- phase 2: attention + modality-sum epilogue ----
    with ExitStack() as ph2:
        scp = ph2.enter_context(tc.tile_pool(name="scp", bufs=4, space="PSUM"))
        avp = ph2.enter_context(tc.tile_pool(name="avp", bufs=1, space="PSUM"))
        etp = ph2.enter_context(tc.tile_pool(name="etp", bufs=5))
        epip = ph2.enter_context(tc.tile_pool(name="epip", bufs=4))
        for qg in range(nqg):
            q0, q1 = qg * QG, (qg + 1) * QG
            ot0s = []
            for m in range(2):
                # one full PSUM bank per qb: matmul start=True clears the
                # whole bank, so accumulation groups must not share one
                avs = [avp.tile([P, DA], F32, padded_shape=[P, 512],
                                tag=f"av{i}", name=f"av{i}")
                       for i in range(nqb)]
                ets = {}

                def emit_av(kb):
                    k0 = kb * DA
                    st = kb == 0
                    sp = kb == nkb - 1
                    et = ets.pop(kb)
                    for qb in range(nqb):
                        nc.tensor.matmul(avs[qb][:],
                                         et[:, qb * P:(qb + 1) * P],
                                         v_sb[m][:, k0:k0 + DA],
                                         start=st, stop=sp)

                for kb in range(nkb):
                    k0 = kb * P
                    sc_ps = scp.tile([P, QG], F32, tag="sc")
                    nc.tensor.matmul(sc_ps[:], kt0[m][:, k0:k0 + P],
                                     qt0[m][:, q0:q1], start=True, stop=False)
                    nc.tensor.matmul(sc_ps[:], kt1[m][:, k0:k0 + P],
                                     qt1[m][:, q0:q1], start=False, stop=True)
                    et = etp.tile([P, QG], BF16, tag="et")
                    nc.scalar.activation(et[:], sc_ps[:], AF.Exp)
                    ets[kb] = et
                    # lag 2 so the AV never waits on the exp latency chain
                    if kb >= 2:
                        emit_av(kb - 2)
                if nkb >= 2:
                    emit_av(nkb - 2)
                emit_av(nkb - 1)

                # drain the avp banks (recip+mult read PSUM) before the
                # adds/stores, so the next unit's AV accumulation can
                # reuse the banks as early as possible
                tmps = []
                last_unit = m == 1 and qg == nqg - 1
                for qb in range(nqb):
                    pp = avs[qb]
                    rc = epip.tile([P, 1], F32, tag=f"rc{m}", name=f"rc{m}")
                    nc.vector.reciprocal(rc[:], pp[:, D:DA])
                    dst = epip.tile([P, D], F32, tag=f"o{m}_{qb}",
                                    name=f"o{m}_{qb}")
                    if last_unit:
                        # ScalarE is done with exps here; pipelines the
                        # tail-exposed epilogue against the DVE recips/adds
                        nc.scalar.activation(dst[:], pp[:, 0:D], AF.Copy,
                                             scale=rc[:])
                    else:
                        nc.vector.tensor_scalar(dst[:], pp[:, 0:D], rc[:],
                                                None, ALU.mult)
                    (ot0s if m == 0 else tmps).append(dst)
                if m == 1:
                    for qb in range(nqb):
                        ot = epip.tile([P, D], BF16, tag="ot", name="ot")
                        nc.vector.tensor_tensor(ot[:], ot0s[qb][:],
                                                tmps[qb][:], ALU.add)
                        r0 = q0 + qb * P
                        nc.sync.dma_start(out=out_ext[r0:r0 + P, :], in_=ot[:])


_CACHE = {}


def _build(s_len=S):
    key = s_len
    if key not in _CACHE:
        nc = bacc.Bacc("TRN2", target_bir_lowering=False, debug=False,
                       num_devices=NCORES)
        xta_ext = nc.dram_tensor("xta", [2 * P, s_len], BF16,
                                 kind="ExternalInput")
        xtb_ext = nc.dram_tensor("xtb", [2 * P, s_len], BF16,
                                 kind="ExternalInput")
        w_ext = nc.dram_tensor("w", [P, 2 * WCOL], BF16, kind="ExternalInput")
        out_ext = nc.dram_tensor("out", [s_len // 2, D], BF16,
                                 kind="ExternalOutput")
        exts = ((xta_ext, xtb_ext), w_ext, out_ext)
        with tile.TileContext(nc) as tc:
            with ExitStack() as ctx:
                _emit(ctx, tc, nc, exts, s_len)
        nc.compile()
        _CACHE[key] = nc
    return _CACHE[key]


def _prep_in_maps(m1, m2, Wq, bq, Wk, bk, Wv, bv, Wo, bo, s_len=S):
    sc = np.float32(1.0 / np.sqrt(D))
    wvo = (Wv.T @ Wo.T).astype(np.float32)           # x @ wvo == (x@Wv.T)@Wo.T
    bvo = (bv @ Wo.T + 0.5 * bo).astype(np.float32)  # per-modality bo/2
    wpack = np.zeros((DA, WCOL), np.float32)
    wpack[:D, 0:D] = Wq.T * sc
    wpack[D, 0:D] = bq * sc
    wpack[:D, D:2 * D] = Wk.T
    wpack[D, D:2 * D] = bk
    wpack[:D, 2 * D:2 * D + D] = wvo
    wpack[D, 2 * D:2 * D + D] = bvo
    wpack[D, 2 * D + D] = 1.0                  # ones col of V2 (sumexp L)
    # one 128-partition tensor: cols 0:WCOL = rows 0:128, cols WCOL: = rows
    # 128:201 (padded) — only 128-partition DMAs fan out over 16 engines
    w_np = np.zeros((P, 2 * WCOL), np.float32)
    w_np[:, 0:WCOL] = wpack[:P]
    w_np[0:DA - P, WCOL:2 * WCOL] = wpack[P:]
    w_np = w_np.astype(NP_BF16)
    half = s_len // 2
    in_maps = []
    for c in range(NCORES):
        b, h = c // 2, c % 2
        r = h * half
        maps = {}
        for key, src in (("xta", m1), ("xtb", m2)):
            x = np.asarray(src[b][:s_len], np.float32)
            xt = np.zeros((2 * P, s_len), np.float32)
            xt[:D] = x.T
            xt[D] = 1.0
            if r:
                xt = np.concatenate([xt[:, r:], xt[:, :r]], axis=1)
            maps[key] = np.ascontiguousarray(xt).astype(NP_BF16)
        maps["w"] = w_np
        in_maps.append(maps)
    return in_maps


def _run(inputs, s_len=S, use_rs=None, trace=False, tmpdir=None):
    m1 = np.asarray(inputs["modal1_input"], np.float32)
    m2 = np.asarray(inputs["modal2_input"], np.float32)
    args = [np.asarray(inputs[k], np.float32)
            for k in ("Wq", "bq", "Wk", "bk", "Wv", "bv", "Wo", "bo")]
    nc = _build(s_len)
    in_maps = _prep_in_maps(m1, m2, *args, s_len=s_len)
    kr = run_bass_kernel_spmd(nc, in_maps, core_ids=list(range(NCORES)),
                              trace=trace, tmpdir=tmpdir)
    res = kr.results
    half = s_len // 2
    out = np.empty((B, s_len, D), np.float32)
    for b in range(B):
        for h in range(2):
            out[b, h * half:(h + 1) * half] = \
                np.asarray(res[2 * b + h]["out"], np.float32)
    return out, kr


def kernel(**inputs):
    out, _ = _run(inputs)
    return out
